# revision 23
# baseline (speedup 1.0000x reference)
"""Expert-parallel MoE (top-2 of 16 experts) for Trainium2, 8 NeuronCores.

Sharding strategy (per spec sharding_hint): expert-parallel, 2 experts per
core in 2 slots (rank-block assignment: the 8 largest experts land in slot
0, the 8 smallest in slot 1, so each slot's compiled capacity hugs the max
actual token count of its octile; capacities are padded to a multiple of 4
only — matmul free dims need no 128-alignment). The router (a [T,16]
matmul + top-2, ~0.01% of total FLOPs) runs on the host at input-shard
time; the "all-to-all token dispatch" is realized as the host-side gather
that builds each core's token batch, and the top-2 weighted combine is the
host-side scatter-add at unshard time. (The code also supports n_slots=4
half-expert chunks; measured slower on HW — each extra matmul instruction
carries ~25-60ns of unhidden overhead, outweighing the smaller padding.)

On-device per core (all heavy FLOPs + memory traffic), per slot with
X_e^T [H, C] (tokens on the PE free dim):
    G1 = gate_w[:, :2048]^T-tiles @ X^T      (PSUM f32)
    G2 = silu(gate_w[:, 2048:] @ X^T)        (ScalarE silu from PSUM)
    HH = G2 * G1 * (up_w @ X^T)              (VectorE, fp16)
    Y^T = down_w-tiles @ HH                  (PSUM f32 -> SBUF -> HBM)

All matmuls in fp16 (weights cast at shard time), f32 accumulation.
Activations are laid out transposed ([H, C], tokens on the moving/free dim)
so every weight matrix is used in its natural [K, M] layout with zero
on-device transposes.

Perf notes (measured on HW):
- fp8 (e4m3) is numerically out of reach: one fp8 matmul already gives
  3.7e-2 rel err vs the 2e-2 budget (measured), so DoubleRow 2x is out.
- Matmul moving free dim is capped at 512 (walrus ISA check
  s3d3_mm_num_elements), so the 2560 matmuls/core are the minimum for
  caps (1112, 1020); each carries ~27ns of unhidden LDWEIGHTS/dispatch.
- walrus does not dedup back-to-back same-stationary LDWEIGHTS; chunk_all
  groups all c-tiles per weight tile anyway (fewest PSUM round-trips).
- y is written in fp16 (half the writeback DMA, ~2e-4 extra rounding).
- warm-up matmuls run during the initial DMA fill so the HAM clock gate
  is already at 2.4 GHz when the first real matmul issues.
- first-slot slabs load as coarse contiguous k-slices split across the
  two HWDGE queues (SP + ACT); fine/strided slices cost 2-5x in DGE
  issue time and starve the head.
"""

import os

import numpy as np

import concourse.tile as tile
import concourse.mybir as mybir
from concourse import bacc
from concourse import bass_utils

N_CORES = 8
N_SLOTS = 2  # expert chunks per core (2 = one whole expert per slot)
E = 16
H = 1024
I_G = 4096  # gate projection width
I_H = 2048  # up/down inner width
KB_H = H // 128  # 8 k-tiles for H-contraction
KB_I = I_H // 128  # 16 k-tiles for I_H-contraction

# 16-bit matmul dtype: fp16 and bf16 run at the same PE rate (1 cyc/row);
# fp16's 10 mantissa bits give ~4x lower rounding error for this data
# (all values well inside fp16 range).
BF16 = mybir.dt.float16
F32 = mybir.dt.float32
NP_BF16 = np.float16
# y output in fp16: halves the writeback DMA and doubles DVE copy rate;
# adds only ~2e-4 RMS rounding on the final output (well inside budget).
Y_DT = mybir.dt.float16


def _ceil_mult(n: int, m: int) -> int:
    return ((n + m - 1) // m) * m


def _round_mant(a: np.ndarray, bits: int) -> np.ndarray:
    """Round fp16 values to `bits` stored mantissa bits (round-to-nearest,
    carry into the exponent is correct rounding).  The zeroed low mantissa
    bits cut PE multiplier partial-product and xbus toggle power, which
    matters because sustained bursts P0-throttle the PE clock; numerics
    stay far inside the 2e-2 budget (bits=6 -> ~0.25% per element)."""
    if not bits or bits >= 10:
        return a
    assert a.dtype == np.float16
    keep = 10 - bits
    v = a.view(np.uint16).astype(np.uint32)
    v = ((v + (1 << (keep - 1))) >> keep) << keep
    # guard: +inf overflow from rounding up the largest normals is
    # impossible here (values << fp16 max), so no special-casing.
    return v.astype(np.uint16).view(np.float16)


def _split_c(C: int):
    """Split capacity C (multiple of 4) into PE free-dim tiles.

    Tiles are kept <=512 (one PSUM bank in f32) and, where possible, >=240
    so the per-tile LDWEIGHTS (~53ns with FWL) stays hidden under the
    matmul stream. Returns list of (offset, width)."""
    assert C % 4 == 0 and C > 0
    q, r = divmod(C, 512)
    style = _OPT.get("split", "rebal")
    if r == 0:
        widths = [512] * q
    elif q == 0 or r >= 240 or style == "tail":
        widths = [512] * q + [r]
    elif style == "b384":
        widths = [512] * (q - 1) + [384, 128 + r]
    else:  # "rebal"
        # rebalance the last full tile with the remainder: two tiles of
        # (512+r)/2 each, keeping every tile >= 240 wide.
        a = ((512 + r) // 2 + 3) // 4 * 4
        widths = [512] * (q - 1) + [a, 512 + r - a]
    out = []
    off = 0
    for w in widths:
        out.append((off, w))
        off += w
    assert off == C and all(wd <= 512 for wd in widths)
    return out


def _chunk2(seq):
    if _OPT["chunk_all"]:
        return [list(seq)]
    return [seq[i : i + 2] for i in range(0, len(seq), 2)]


_OPT = dict(
    psum_merged=True, head_split=8, wp_bufs=6, tp_bufs=4, yp_bufs=3,
    chunk_all=True, split="rebal",
    # PE warm-up: dummy matmuls issued during the initial DMA fill so the
    # HAM clock-gate reaches 2.4 GHz before the first real matmul (the
    # inter-rep gap exceeds the ~3.4us MID window, so each rep starts cold).
    warm_n=14, warm_w=256,
    # first-half slabs issue as coarse k-slices split across both HWDGE
    # queues (xs+sg1 on SP, sg2+su on ACT) so the head isn't issue-bound.
    colslice_first=True,
    # down-phase PSUM drains alternate DVE/ACT and y DMAs go per c-tile,
    # shortening the exposed tail after the last matmul.
    alt_copy=True, y_split=True,
    # round host-cast fp16 operands to this many stored mantissa bits
    # (0 = off) to cut PE toggle power and soften the P0 burst throttle.
    mant_bits=6,
)
if os.environ.get("MOE_OPT"):
    import json as _json

    _OPT.update(_json.loads(os.environ["MOE_OPT"]))


def _load_slab(nc, pool, shape, src, tag, name, parts=2):
    """Allocate a [128, kb, n] slab and load it with `parts` DMAs split
    along the k dimension, so the first k-tiles land early and the PE can
    start before the whole slab arrives."""
    t = pool.tile(shape, BF16, tag=tag, name=name)
    kb = shape[1]
    step = max(1, kb // parts)
    for a in range(0, kb, step):
        b = min(kb, a + step)
        nc.sync.dma_start(out=t[:, a:b, :], in_=src[:, a:b, :])
    return t


def _ptag(name):
    return "ps" if _OPT["psum_merged"] else name


def _expert_ffn(nc, wp, xp, hp, yp, tp, pp, x, g, u, d, y, C, CMAX, first=False):
    """Emit one slot's FFN: y[H, C] = down( silu(g2)*g1*up ) for x[H, C]."""
    ct = _split_c(C)

    xr = x.rearrange("(kb p) c -> p kb c", p=128)  # [128, 8, C]
    gr = g.rearrange("(kb p) i -> p kb i", p=128)  # [128, 8, 4096]
    ur = u.rearrange("(kb p) i -> p kb i", p=128)  # [128, 8, 2048]
    dr = d.rearrange("(kb p) h -> p kb h", p=128)  # [128, 16, 1024]
    yr = y.rearrange("(hb p) c -> p hb c", p=128)  # [128, 8, C]

    hs_parts = _OPT["head_split"] if first else 1
    xs = xp.tile([128, KB_H, CMAX], BF16, tag="xt", name="xs")
    sg2_0 = sg1_0 = su_0 = None
    if first and _OPT["colslice_first"]:
        # First slot, first half: feed the PE's first accumulation chains
        # with the minimum data prefix.  Weight-column slices issue on the
        # ACT HWDGE queue, xs on the SP queue (two queues issue in
        # parallel); il=0 is processed c-chunked ([c0] then the rest), so
        # only xs[:, :, :c0] + the il0 weight columns gate the PE start.
        sg2_0 = wp.tile([128, KB_H, 1024], BF16, tag="w", name="sg2")
        sg1_0 = wp.tile([128, KB_H, 1024], BF16, tag="w", name="sg1")
        su_0 = wp.tile([128, KB_H, 1024], BF16, tag="w", name="su")
        srcs = (
            (sg2_0, gr[:, :, 2048 : 2048 + 1024]),
            (sg1_0, gr[:, :, 0:1024]),
            (su_0, ur[:, :, 0:1024]),
        )
        # Coarse contiguous k-slices only (fine/strided slices cost 2-5x in
        # DGE issue time).  xs + sg1 on the SP queue, sg2 + su on the ACT
        # queue: both queues issue in parallel, halving the head's
        # issue-serialization, and each stream's k0 lands early.
        for k in range(KB_H):
            nc.scalar.dma_start(
                out=sg2_0[:, k : k + 1, :], in_=srcs[0][1][:, k : k + 1, :]
            )
            nc.sync.dma_start(out=xs[:, k : k + 1, :C], in_=xr[:, k : k + 1, :])
        for a in range(0, KB_H, 2):
            nc.sync.dma_start(
                out=sg1_0[:, a : a + 2, :], in_=srcs[1][1][:, a : a + 2, :]
            )
            nc.scalar.dma_start(
                out=su_0[:, a : a + 2, :], in_=srcs[2][1][:, a : a + 2, :]
            )
    elif hs_parts > 1:
        # First slot: the PE's very first LDW/MM needs sg2[k=0] and
        # xs[k=0]. Interleave their k-slice DMAs so the earliest-needed
        # pieces land on distinct queues in the first round-robin wave.
        sg2_0 = wp.tile([128, KB_H, 1024], BF16, tag="w", name="sg2")
        for k in range(KB_H):
            nc.sync.dma_start(
                out=sg2_0[:, k : k + 1, :],
                in_=gr[:, k : k + 1, 2048 : 2048 + 1024],
            )
            nc.sync.dma_start(out=xs[:, k : k + 1, :C], in_=xr[:, k : k + 1, :])
    else:
        nc.sync.dma_start(out=xs[:, :, :C], in_=xr)

    hh = hp.tile([128, KB_I, CMAX], BF16, tag="hh", name="hh")

    # ---- gate + up fused phase ----
    for half in range(2):  # hh i-tiles 0-7 / 8-15
        lo = half * 1024
        p = hs_parts if half == 0 else 1
        if half == 0 and sg2_0 is not None:
            sg2 = sg2_0
        else:
            sg2 = _load_slab(
                nc, wp, [128, KB_H, 1024],
                gr[:, :, 2048 + lo : 2048 + lo + 1024], "w", "sg2", parts=p,
            )
        if half == 0 and sg1_0 is not None:
            sg1, su = sg1_0, su_0
        else:
            sg1 = _load_slab(
                nc, wp, [128, KB_H, 1024], gr[:, :, lo : lo + 1024], "w", "sg1",
                parts=p,
            )
            su = _load_slab(
                nc, wp, [128, KB_H, 1024], ur[:, :, lo : lo + 1024], "w", "su",
                parts=p,
            )

        for il in range(8):
            i = half * 8 + il
            ms = slice(il * 128, (il + 1) * 128)
            for cc in _chunk2(ct):
                # --- g2 stream (silu half) ---
                pg2 = [
                    pp.tile([128, w], F32, tag=_ptag("pg2"), name="pg2") for (_, w) in cc
                ]
                for k in range(KB_H):
                    for j, (off, w) in enumerate(cc):
                        nc.tensor.matmul(
                            pg2[j],
                            sg2[:, k, ms],
                            xs[:, k, off : off + w],
                            start=(k == 0),
                            stop=(k == KB_H - 1),
                        )
                sil = []
                for j, (off, w) in enumerate(cc):
                    t = tp.tile([128, 512], BF16, tag="t", name="t")
                    nc.scalar.activation(
                        out=t[:, :w],
                        in_=pg2[j],
                        func=mybir.ActivationFunctionType.Silu,
                    )
                    sil.append(t)
                # --- g1 stream ---
                pg1 = [
                    pp.tile([128, w], F32, tag=_ptag("pg1"), name="pg1") for (_, w) in cc
                ]
                for k in range(KB_H):
                    for j, (off, w) in enumerate(cc):
                        nc.tensor.matmul(
                            pg1[j],
                            sg1[:, k, ms],
                            xs[:, k, off : off + w],
                            start=(k == 0),
                            stop=(k == KB_H - 1),
                        )
                g12 = []
                for j, (off, w) in enumerate(cc):
                    t2 = tp.tile([128, 512], BF16, tag="g12", name="t2")
                    nc.vector.tensor_mul(t2[:, :w], sil[j][:, :w], pg1[j])
                    g12.append(t2)
                # --- up stream ---
                pu = [
                    pp.tile([128, w], F32, tag=_ptag("pu"), name="pu") for (_, w) in cc
                ]
                for k in range(KB_H):
                    for j, (off, w) in enumerate(cc):
                        nc.tensor.matmul(
                            pu[j],
                            su[:, k, ms],
                            xs[:, k, off : off + w],
                            start=(k == 0),
                            stop=(k == KB_H - 1),
                        )
                for j, (off, w) in enumerate(cc):
                    nc.vector.tensor_mul(
                        hh[:, i, off : off + w], g12[j][:, :w], pu[j]
                    )

    # ---- down phase ----
    dlo = _load_slab(nc, wp, [128, KB_H, 1024], dr[:, 0:8, :], "w", "dlo")
    dhi = _load_slab(nc, wp, [128, KB_H, 1024], dr[:, 8:16, :], "w", "dhi")

    for h in range(8):
        ms = slice(h * 128, (h + 1) * 128)
        yl = yp.tile([128, CMAX], Y_DT, tag="y", name="yl")
        for cc in _chunk2(ct):
            pd = [
                pp.tile([128, w], F32, tag=_ptag("pd"), name="pd") for (_, w) in cc
            ]
            for k in range(KB_I):
                sl = dlo if k < 8 else dhi
                for j, (off, w) in enumerate(cc):
                    nc.tensor.matmul(
                        pd[j],
                        sl[:, k % 8, ms],
                        hh[:, k, off : off + w],
                        start=(k == 0),
                        stop=(k == KB_I - 1),
                    )
            for j, (off, w) in enumerate(cc):
                if _OPT["alt_copy"] and j % 2 == 1:
                    nc.scalar.copy(yl[:, off : off + w], pd[j])
                else:
                    nc.vector.tensor_copy(yl[:, off : off + w], pd[j])
                if _OPT["y_split"]:
                    ww = min(off + w, C) - off
                    if ww > 0:
                        eng = nc.scalar if j % 2 == 1 else nc.sync
                        eng.dma_start(
                            out=yr[:, h, off : off + ww],
                            in_=yl[:, off : off + ww],
                        )
        if not _OPT["y_split"]:
            nc.sync.dma_start(out=yr[:, h, :], in_=yl[:, :C])


def _build_nc(caps, reps: int = 1):
    """Build + compile the N-slot SPMD program (same on all cores)."""
    caps = tuple(caps)
    CMAX = max(caps)
    nc = bacc.Bacc(
        "TRN2", target_bir_lowering=False, debug=False, num_devices=N_CORES
    )
    dram = {}
    for s, C in enumerate(caps):
        dram[f"x{s}"] = nc.dram_tensor(
            f"x{s}", [H, C], BF16, kind="ExternalInput"
        ).ap()
        dram[f"g{s}"] = nc.dram_tensor(
            f"g{s}", [H, I_G], BF16, kind="ExternalInput"
        ).ap()
        dram[f"u{s}"] = nc.dram_tensor(
            f"u{s}", [H, I_H], BF16, kind="ExternalInput"
        ).ap()
        dram[f"d{s}"] = nc.dram_tensor(
            f"d{s}", [I_H, H], BF16, kind="ExternalInput"
        ).ap()
        dram[f"y{s}"] = nc.dram_tensor(
            f"y{s}", [H, C], Y_DT, kind="ExternalOutput"
        ).ap()

    with tile.TileContext(nc) as tc:
        with (
            tc.tile_pool(name="wp", bufs=_OPT["wp_bufs"]) as wp,  # 16KB/part weight slabs
            tc.tile_pool(name="xp", bufs=2) as xp,
            tc.tile_pool(name="hp", bufs=1) as hp,
            tc.tile_pool(name="yp", bufs=_OPT["yp_bufs"]) as yp,
            tc.tile_pool(name="tp", bufs=_OPT["tp_bufs"]) as tp,
            tc.tile_pool(name="pp", bufs=(8 if _OPT["psum_merged"] else 2), space="PSUM") as pp,
        ):

            def body():
                if _OPT["warm_n"]:
                    # PE clock warm-up during the initial DMA fill: the HAM
                    # gate needs ~3.4us of sustained PE activity to pass
                    # 2.4 GHz; these dummy matmuls cost idle time only.
                    ww = _OPT["warm_w"]
                    wmt = xp.tile([128, ww], BF16, tag="warm", name="warm")
                    nc.vector.memset(wmt, 0)
                    pw = pp.tile([128, ww], F32, tag=_ptag("pw"), name="pw")
                    for _ in range(_OPT["warm_n"]):
                        nc.tensor.matmul(
                            pw, wmt[:, :128], wmt, start=True, stop=True
                        )
                for s, C in enumerate(caps):
                    _expert_ffn(
                        nc,
                        wp,
                        xp,
                        hp,
                        yp,
                        tp,
                        pp,
                        dram[f"x{s}"],
                        dram[f"g{s}"],
                        dram[f"u{s}"],
                        dram[f"d{s}"],
                        dram[f"y{s}"],
                        C,
                        CMAX,
                        first=(s == 0),
                    )

            if reps == 1:
                body()
            else:
                ET = mybir.EngineType
                with tc.For_i(
                    0,
                    reps,
                    1,
                    hint_engines=(ET.PE, ET.DVE, ET.Activation, ET.SP, ET.Pool),
                ):
                    body()

    nc.compile()
    return nc


_NC_CACHE: dict = {}


def _get_nc(caps):
    key = tuple(caps)
    if key not in _NC_CACHE:
        _NC_CACHE[key] = _build_nc(key)
    return _NC_CACHE[key]


def _route_and_shard(hs, rw, gw, uw, dw, n_slots=N_SLOTS):
    """Host-side router + expert-parallel sharding of the full inputs."""
    B, S, _ = hs.shape
    T = B * S
    x = np.ascontiguousarray(hs.reshape(T, H)).astype(np.float32, copy=False)

    logits = x @ rw.astype(np.float32)  # [T, E]
    order = np.argsort(-logits, axis=1, kind="stable")[:, :2]
    l12 = np.take_along_axis(logits, order, axis=1).astype(np.float64)
    w1 = 1.0 / (1.0 + np.exp(l12[:, 1] - l12[:, 0]))  # renormalized top-2
    w2 = 1.0 - w1
    i1, i2 = order[:, 0], order[:, 1]

    idx, wts = [], []
    for e in range(E):
        m1 = i1 == e
        ide = np.nonzero(m1 | (i2 == e))[0]
        we = np.where(m1[ide], w1[ide], w2[ide]).astype(np.float32)
        idx.append(ide)
        wts.append(we)
    counts = np.array([len(v) for v in idx])

    # Split each expert's tokens into (n_slots/2) equal chunks -> 8*n_slots
    # chunks; assign the j-th largest octile of chunks to slot j of the 8
    # cores, so each slot's capacity hugs its octile's max and padded
    # compute is minimized.
    pieces = max(1, n_slots // 2)  # E=16 experts over 8 cores: n_slots/2 pieces each
    chunks = []  # (size, expert, start)
    for e in range(E):
        n = int(counts[e])
        st = 0
        for p in range(pieces):
            sz = (n - st + (pieces - p - 1)) // (pieces - p)  # even split
            chunks.append((sz, e, st))
            st += sz
        assert st == n
    chunks.sort(key=lambda t: (-t[0], t[1], t[2]))
    assert len(chunks) == N_CORES * n_slots

    caps = []
    for j in range(n_slots):
        mx = max(sz for sz, _, _ in chunks[N_CORES * j : N_CORES * (j + 1)])
        caps.append(max(64, _ceil_mult(mx, 4)))
    slots = [
        [chunks[N_CORES * j + c] for j in range(n_slots)] for c in range(N_CORES)
    ]

    mb = _OPT.get("mant_bits", 0)
    xT = _round_mant(np.ascontiguousarray(x.T).astype(NP_BF16), mb)  # [H, T]

    in_maps = []
    for c in range(N_CORES):
        m = {}
        for s, (sz, e, st) in enumerate(slots[c]):
            xe = np.zeros((H, caps[s]), NP_BF16)
            xe[:, :sz] = xT[:, idx[e][st : st + sz]]
            m[f"x{s}"] = xe
            m[f"g{s}"] = _round_mant(np.ascontiguousarray(gw[e]).astype(NP_BF16), mb)
            m[f"u{s}"] = _round_mant(np.ascontiguousarray(uw[e]).astype(NP_BF16), mb)
            m[f"d{s}"] = _round_mant(np.ascontiguousarray(dw[e]).astype(NP_BF16), mb)
        in_maps.append(m)

    meta = dict(
        B=B, S=S, T=T, idx=idx, wts=wts, counts=counts,
        slots=slots, caps=tuple(caps),
    )
    return in_maps, meta


def _combine(results, meta):
    """Host-side top-2 weighted combine (unshard)."""
    T = meta["T"]
    out = np.zeros((T, H), np.float32)
    for c in range(N_CORES):
        for s, (sz, e, st) in enumerate(meta["slots"][c]):
            if sz == 0:
                continue
            y = results[c][f"y{s}"][:, :sz].astype(np.float32)  # [H, sz]
            ide = meta["idx"][e][st : st + sz]
            out[ide] += meta["wts"][e][st : st + sz, None] * y.T
    return out.reshape(meta["B"], meta["S"], H)


def _run_spmd(nc, in_maps):
    try:
        return bass_utils.run_bass_kernel_spmd(
            nc, in_maps, core_ids=list(range(N_CORES))
        )
    except ModuleNotFoundError:
        # axon NTFF profiling hook unavailable in this container; retry
        # with tracing force-disabled.
        os.environ["BASS_NEVER_TRACE"] = "1"
        try:
            return bass_utils.run_bass_kernel_spmd(
                nc, in_maps, core_ids=list(range(N_CORES))
            )
        finally:
            os.environ.pop("BASS_NEVER_TRACE", None)


def kernel(hidden_states, router_w, gate_w, up_w, down_w):
    hs = np.asarray(hidden_states)
    rw = np.asarray(router_w)
    gw = np.asarray(gate_w)
    uw = np.asarray(up_w)
    dw = np.asarray(down_w)

    in_maps, meta = _route_and_shard(hs, rw, gw, uw, dw)
    nc = _get_nc(meta["caps"])
    res = _run_spmd(nc, in_maps)
    return _combine(res.results, meta)



# revision 26
# speedup vs baseline: 1.0261x; 1.0261x over previous
"""Expert-parallel MoE (top-2 of 16 experts) for Trainium2, 8 NeuronCores.

Sharding strategy (per spec sharding_hint): expert-parallel, 2 experts per
core in 2 slots (rank-block assignment: the 8 largest experts land in slot
0, the 8 smallest in slot 1, so each slot's compiled capacity hugs the max
actual token count of its octile; capacities are padded to a multiple of 4
only — matmul free dims need no 128-alignment). The router (a [T,16]
matmul + top-2, ~0.01% of total FLOPs) runs on the host at input-shard
time; the "all-to-all token dispatch" is realized as the host-side gather
that builds each core's token batch, and the top-2 weighted combine is the
host-side scatter-add at unshard time. (The code also supports n_slots=4
half-expert chunks; measured slower on HW — each extra matmul instruction
carries ~25-60ns of unhidden overhead, outweighing the smaller padding.)

On-device per core (all heavy FLOPs + memory traffic), per slot with
X_e^T [H, C] (tokens on the PE free dim):
    G1 = gate_w[:, :2048]^T-tiles @ X^T      (PSUM f32)
    G2 = silu(gate_w[:, 2048:] @ X^T)        (ScalarE silu from PSUM)
    HH = G2 * G1 * (up_w @ X^T)              (VectorE, fp16)
    Y^T = down_w-tiles @ HH                  (PSUM f32 -> SBUF -> HBM)

All matmuls in fp16 (weights cast at shard time), f32 accumulation.
Activations are laid out transposed ([H, C], tokens on the moving/free dim)
so every weight matrix is used in its natural [K, M] layout with zero
on-device transposes.

Perf notes (measured on HW):
- fp8 (e4m3) is numerically out of reach: one fp8 matmul already gives
  3.7e-2 rel err vs the 2e-2 budget (measured), so DoubleRow 2x is out.
- Matmul moving free dim is capped at 512 (walrus ISA check
  s3d3_mm_num_elements), so the 2560 matmuls/core are the minimum for
  caps (1112, 1020); each carries ~27ns of unhidden LDWEIGHTS/dispatch.
- walrus does not dedup back-to-back same-stationary LDWEIGHTS; chunk_all
  groups all c-tiles per weight tile anyway (fewest PSUM round-trips).
- y is written in fp16 (half the writeback DMA, ~2e-4 extra rounding).
- warm-up matmuls run during the initial DMA fill so the HAM clock gate
  is already at 2.4 GHz when the first real matmul issues.
- first-slot slabs load as coarse contiguous k-slices split across the
  two HWDGE queues (SP + ACT); fine/strided slices cost 2-5x in DGE
  issue time and starve the head.
"""

import os

import numpy as np

import concourse.tile as tile
import concourse.mybir as mybir
from concourse import bacc
from concourse import bass_utils

N_CORES = 8
N_SLOTS = 2  # expert chunks per core (2 = one whole expert per slot)
E = 16
H = 1024
I_G = 4096  # gate projection width
I_H = 2048  # up/down inner width
KB_H = H // 128  # 8 k-tiles for H-contraction
KB_I = I_H // 128  # 16 k-tiles for I_H-contraction

# 16-bit matmul dtype: fp16 and bf16 run at the same PE rate (1 cyc/row);
# fp16's 10 mantissa bits give ~4x lower rounding error for this data
# (all values well inside fp16 range).
BF16 = mybir.dt.float16
F32 = mybir.dt.float32
NP_BF16 = np.float16
# y output in fp16: halves the writeback DMA and doubles DVE copy rate;
# adds only ~2e-4 RMS rounding on the final output (well inside budget).
Y_DT = mybir.dt.float16


def _ceil_mult(n: int, m: int) -> int:
    return ((n + m - 1) // m) * m


def _round_mant(a: np.ndarray, bits: int) -> np.ndarray:
    """Round fp16 values to `bits` stored mantissa bits (round-to-nearest,
    carry into the exponent is correct rounding).  The zeroed low mantissa
    bits cut PE multiplier partial-product and xbus toggle power, which
    matters because sustained bursts P0-throttle the PE clock; numerics
    stay far inside the 2e-2 budget (bits=6 -> ~0.25% per element)."""
    if not bits or bits >= 10:
        return a
    assert a.dtype == np.float16
    keep = 10 - bits
    v = a.view(np.uint16).astype(np.uint32)
    v = ((v + (1 << (keep - 1))) >> keep) << keep
    # guard: +inf overflow from rounding up the largest normals is
    # impossible here (values << fp16 max), so no special-casing.
    return v.astype(np.uint16).view(np.float16)


def _split_c(C: int):
    """Split capacity C (multiple of 4) into PE free-dim tiles.

    Tiles are kept <=512 (one PSUM bank in f32) and, where possible, >=240
    so the per-tile LDWEIGHTS (~53ns with FWL) stays hidden under the
    matmul stream. Returns list of (offset, width)."""
    assert C % 4 == 0 and C > 0
    q, r = divmod(C, 512)
    style = _OPT.get("split", "rebal")
    if style == "even" and C > 512:
        # even tiles at or below even_w — probes the narrow-N PE fast path
        n = -(-C // _OPT.get("even_w", 384))
        base = (C // n + 3) // 4 * 4
        widths = [base] * (n - 1) + [C - base * (n - 1)]
    elif r == 0:
        widths = [512] * q
    elif q == 0 or r >= 240 or style == "tail":
        widths = [512] * q + [r]
    elif style == "b384":
        widths = [512] * (q - 1) + [384, 128 + r]
    else:  # "rebal"
        # rebalance the last full tile with the remainder: two tiles of
        # (512+r)/2 each, keeping every tile >= 240 wide.
        a = ((512 + r) // 2 + 3) // 4 * 4
        widths = [512] * (q - 1) + [a, 512 + r - a]
    out = []
    off = 0
    for w in widths:
        out.append((off, w))
        off += w
    assert off == C and all(wd <= 512 for wd in widths)
    return out


def _chunk2(seq):
    if _OPT["chunk_all"]:
        return [list(seq)]
    return [seq[i : i + 2] for i in range(0, len(seq), 2)]


_OPT = dict(
    psum_merged=True, head_split=8, wp_bufs=6, tp_bufs=4, yp_bufs=3,
    chunk_all=True, split="rebal",
    # PE warm-up: dummy matmuls issued during the initial DMA fill so the
    # HAM clock-gate reaches 2.4 GHz before the first real matmul (the
    # inter-rep gap exceeds the ~3.4us MID window, so each rep starts cold).
    warm_n=14, warm_w=256,
    # first-half slabs issue as coarse k-slices split across both HWDGE
    # queues (xs+sg1 on SP, sg2+su on ACT) so the head isn't issue-bound.
    colslice_first=True,
    # down-phase PSUM drains alternate DVE/ACT and y DMAs go per c-tile,
    # shortening the exposed tail after the last matmul.
    alt_copy=True, y_split=True,
    # round host-cast fp16 operands to this many stored mantissa bits
    # (0 = off) to cut PE toggle power.  Measured: no reproducible effect
    # on the P0 burst throttle (522 vs 532 us across two runs, straddling
    # the 524-529 no-rounding band), so it stays off — not worth trading
    # the 29x error margin (6.9e-4) down to 2.2x (9.2e-3) for noise.
    mant_bits=0,
)
if os.environ.get("MOE_OPT"):
    import json as _json

    _OPT.update(_json.loads(os.environ["MOE_OPT"]))


def _load_slab(nc, pool, shape, src, tag, name, parts=2):
    """Allocate a [128, kb, n] slab and load it with `parts` DMAs split
    along the k dimension, so the first k-tiles land early and the PE can
    start before the whole slab arrives."""
    t = pool.tile(shape, BF16, tag=tag, name=name)
    kb = shape[1]
    step = max(1, kb // parts)
    for a in range(0, kb, step):
        b = min(kb, a + step)
        nc.sync.dma_start(out=t[:, a:b, :], in_=src[:, a:b, :])
    return t


def _ptag(name):
    return "ps" if _OPT["psum_merged"] else name


def _expert_ffn(nc, wp, xp, hp, yp, tp, pp, x, g, u, d, y, C, CMAX, first=False):
    """Emit one slot's FFN: y[H, C] = down( silu(g2)*g1*up ) for x[H, C]."""
    ct = _split_c(C)

    xr = x.rearrange("(kb p) c -> p kb c", p=128)  # [128, 8, C]
    gr = g.rearrange("(kb p) i -> p kb i", p=128)  # [128, 8, 4096]
    ur = u.rearrange("(kb p) i -> p kb i", p=128)  # [128, 8, 2048]
    dr = d.rearrange("(kb p) h -> p kb h", p=128)  # [128, 16, 1024]
    yr = y.rearrange("(hb p) c -> p hb c", p=128)  # [128, 8, C]

    hs_parts = _OPT["head_split"] if first else 1
    xs = xp.tile([128, KB_H, CMAX], BF16, tag="xt", name="xs")
    sg2_0 = sg1_0 = su_0 = None
    if first and _OPT["colslice_first"]:
        # First slot, first half: feed the PE's first accumulation chains
        # with the minimum data prefix.  Weight-column slices issue on the
        # ACT HWDGE queue, xs on the SP queue (two queues issue in
        # parallel); il=0 is processed c-chunked ([c0] then the rest), so
        # only xs[:, :, :c0] + the il0 weight columns gate the PE start.
        sg2_0 = wp.tile([128, KB_H, 1024], BF16, tag="w", name="sg2")
        sg1_0 = wp.tile([128, KB_H, 1024], BF16, tag="w", name="sg1")
        su_0 = wp.tile([128, KB_H, 1024], BF16, tag="w", name="su")
        srcs = (
            (sg2_0, gr[:, :, 2048 : 2048 + 1024]),
            (sg1_0, gr[:, :, 0:1024]),
            (su_0, ur[:, :, 0:1024]),
        )
        # Coarse contiguous k-slices only (fine/strided slices cost 2-5x in
        # DGE issue time).  xs + sg1 on the SP queue, sg2 + su on the ACT
        # queue: both queues issue in parallel, halving the head's
        # issue-serialization, and each stream's k0 lands early.
        for k in range(KB_H):
            nc.scalar.dma_start(
                out=sg2_0[:, k : k + 1, :], in_=srcs[0][1][:, k : k + 1, :]
            )
            nc.sync.dma_start(out=xs[:, k : k + 1, :C], in_=xr[:, k : k + 1, :])
        for a in range(0, KB_H, 2):
            nc.sync.dma_start(
                out=sg1_0[:, a : a + 2, :], in_=srcs[1][1][:, a : a + 2, :]
            )
            nc.scalar.dma_start(
                out=su_0[:, a : a + 2, :], in_=srcs[2][1][:, a : a + 2, :]
            )
    elif hs_parts > 1:
        # First slot: the PE's very first LDW/MM needs sg2[k=0] and
        # xs[k=0]. Interleave their k-slice DMAs so the earliest-needed
        # pieces land on distinct queues in the first round-robin wave.
        sg2_0 = wp.tile([128, KB_H, 1024], BF16, tag="w", name="sg2")
        for k in range(KB_H):
            nc.sync.dma_start(
                out=sg2_0[:, k : k + 1, :],
                in_=gr[:, k : k + 1, 2048 : 2048 + 1024],
            )
            nc.sync.dma_start(out=xs[:, k : k + 1, :C], in_=xr[:, k : k + 1, :])
    else:
        nc.sync.dma_start(out=xs[:, :, :C], in_=xr)

    hh = hp.tile([128, KB_I, CMAX], BF16, tag="hh", name="hh")

    # ---- gate + up fused phase ----
    for half in range(2):  # hh i-tiles 0-7 / 8-15
        lo = half * 1024
        p = hs_parts if half == 0 else 1
        if half == 0 and sg2_0 is not None:
            sg2 = sg2_0
        else:
            sg2 = _load_slab(
                nc, wp, [128, KB_H, 1024],
                gr[:, :, 2048 + lo : 2048 + lo + 1024], "w", "sg2", parts=p,
            )
        if half == 0 and sg1_0 is not None:
            sg1, su = sg1_0, su_0
        else:
            sg1 = _load_slab(
                nc, wp, [128, KB_H, 1024], gr[:, :, lo : lo + 1024], "w", "sg1",
                parts=p,
            )
            su = _load_slab(
                nc, wp, [128, KB_H, 1024], ur[:, :, lo : lo + 1024], "w", "su",
                parts=p,
            )

        for il in range(8):
            i = half * 8 + il
            ms = slice(il * 128, (il + 1) * 128)
            for cc in _chunk2(ct):
                # --- g2 stream (silu half) ---
                pg2 = [
                    pp.tile([128, w], F32, tag=_ptag("pg2"), name="pg2") for (_, w) in cc
                ]
                for k in range(KB_H):
                    for j, (off, w) in enumerate(cc):
                        nc.tensor.matmul(
                            pg2[j],
                            sg2[:, k, ms],
                            xs[:, k, off : off + w],
                            start=(k == 0),
                            stop=(k == KB_H - 1),
                        )
                sil = []
                for j, (off, w) in enumerate(cc):
                    t = tp.tile([128, 512], BF16, tag="t", name="t")
                    nc.scalar.activation(
                        out=t[:, :w],
                        in_=pg2[j],
                        func=mybir.ActivationFunctionType.Silu,
                    )
                    sil.append(t)
                # --- g1 stream ---
                pg1 = [
                    pp.tile([128, w], F32, tag=_ptag("pg1"), name="pg1") for (_, w) in cc
                ]
                for k in range(KB_H):
                    for j, (off, w) in enumerate(cc):
                        nc.tensor.matmul(
                            pg1[j],
                            sg1[:, k, ms],
                            xs[:, k, off : off + w],
                            start=(k == 0),
                            stop=(k == KB_H - 1),
                        )
                g12 = []
                for j, (off, w) in enumerate(cc):
                    t2 = tp.tile([128, 512], BF16, tag="g12", name="t2")
                    nc.vector.tensor_mul(t2[:, :w], sil[j][:, :w], pg1[j])
                    g12.append(t2)
                # --- up stream ---
                pu = [
                    pp.tile([128, w], F32, tag=_ptag("pu"), name="pu") for (_, w) in cc
                ]
                for k in range(KB_H):
                    for j, (off, w) in enumerate(cc):
                        nc.tensor.matmul(
                            pu[j],
                            su[:, k, ms],
                            xs[:, k, off : off + w],
                            start=(k == 0),
                            stop=(k == KB_H - 1),
                        )
                for j, (off, w) in enumerate(cc):
                    nc.vector.tensor_mul(
                        hh[:, i, off : off + w], g12[j][:, :w], pu[j]
                    )

    # ---- down phase ----
    dlo = _load_slab(nc, wp, [128, KB_H, 1024], dr[:, 0:8, :], "w", "dlo")
    dhi = _load_slab(nc, wp, [128, KB_H, 1024], dr[:, 8:16, :], "w", "dhi")

    for h in range(8):
        ms = slice(h * 128, (h + 1) * 128)
        yl = yp.tile([128, CMAX], Y_DT, tag="y", name="yl")
        for cc in _chunk2(ct):
            pd = [
                pp.tile([128, w], F32, tag=_ptag("pd"), name="pd") for (_, w) in cc
            ]
            for k in range(KB_I):
                sl = dlo if k < 8 else dhi
                for j, (off, w) in enumerate(cc):
                    nc.tensor.matmul(
                        pd[j],
                        sl[:, k % 8, ms],
                        hh[:, k, off : off + w],
                        start=(k == 0),
                        stop=(k == KB_I - 1),
                    )
            for j, (off, w) in enumerate(cc):
                if _OPT["alt_copy"] and j % 2 == 1:
                    nc.scalar.copy(yl[:, off : off + w], pd[j])
                else:
                    nc.vector.tensor_copy(yl[:, off : off + w], pd[j])
                if _OPT["y_split"]:
                    ww = min(off + w, C) - off
                    if ww > 0:
                        eng = nc.scalar if j % 2 == 1 else nc.sync
                        eng.dma_start(
                            out=yr[:, h, off : off + ww],
                            in_=yl[:, off : off + ww],
                        )
        if not _OPT["y_split"]:
            nc.sync.dma_start(out=yr[:, h, :], in_=yl[:, :C])


def _build_nc(caps, reps: int = 1):
    """Build + compile the N-slot SPMD program (same on all cores)."""
    caps = tuple(caps)
    CMAX = max(caps)
    nc = bacc.Bacc(
        "TRN2", target_bir_lowering=False, debug=False, num_devices=N_CORES
    )
    dram = {}
    for s, C in enumerate(caps):
        dram[f"x{s}"] = nc.dram_tensor(
            f"x{s}", [H, C], BF16, kind="ExternalInput"
        ).ap()
        dram[f"g{s}"] = nc.dram_tensor(
            f"g{s}", [H, I_G], BF16, kind="ExternalInput"
        ).ap()
        dram[f"u{s}"] = nc.dram_tensor(
            f"u{s}", [H, I_H], BF16, kind="ExternalInput"
        ).ap()
        dram[f"d{s}"] = nc.dram_tensor(
            f"d{s}", [I_H, H], BF16, kind="ExternalInput"
        ).ap()
        dram[f"y{s}"] = nc.dram_tensor(
            f"y{s}", [H, C], Y_DT, kind="ExternalOutput"
        ).ap()

    with tile.TileContext(nc) as tc:
        with (
            tc.tile_pool(name="wp", bufs=_OPT["wp_bufs"]) as wp,  # 16KB/part weight slabs
            tc.tile_pool(name="xp", bufs=2) as xp,
            tc.tile_pool(name="hp", bufs=1) as hp,
            tc.tile_pool(name="yp", bufs=_OPT["yp_bufs"]) as yp,
            tc.tile_pool(name="tp", bufs=_OPT["tp_bufs"]) as tp,
            tc.tile_pool(name="pp", bufs=(8 if _OPT["psum_merged"] else 2), space="PSUM") as pp,
        ):

            def body():
                if _OPT["warm_n"]:
                    # PE clock warm-up during the initial DMA fill: the HAM
                    # gate needs ~3.4us of sustained PE activity to pass
                    # 2.4 GHz; these dummy matmuls cost idle time only.
                    ww = _OPT["warm_w"]
                    wmt = xp.tile([128, ww], BF16, tag="warm", name="warm")
                    nc.vector.memset(wmt, 0)
                    pw = pp.tile([128, ww], F32, tag=_ptag("pw"), name="pw")
                    for _ in range(_OPT["warm_n"]):
                        nc.tensor.matmul(
                            pw, wmt[:, :128], wmt, start=True, stop=True
                        )
                for s, C in enumerate(caps):
                    _expert_ffn(
                        nc,
                        wp,
                        xp,
                        hp,
                        yp,
                        tp,
                        pp,
                        dram[f"x{s}"],
                        dram[f"g{s}"],
                        dram[f"u{s}"],
                        dram[f"d{s}"],
                        dram[f"y{s}"],
                        C,
                        CMAX,
                        first=(s == 0),
                    )

            if reps == 1:
                body()
            else:
                ET = mybir.EngineType
                with tc.For_i(
                    0,
                    reps,
                    1,
                    hint_engines=(ET.PE, ET.DVE, ET.Activation, ET.SP, ET.Pool),
                ):
                    body()

    nc.compile()
    return nc


_NC_CACHE: dict = {}


def _get_nc(caps):
    key = tuple(caps)
    if key not in _NC_CACHE:
        _NC_CACHE[key] = _build_nc(key)
    return _NC_CACHE[key]


def _route_and_shard(hs, rw, gw, uw, dw, n_slots=N_SLOTS):
    """Host-side router + expert-parallel sharding of the full inputs."""
    B, S, _ = hs.shape
    T = B * S
    x = np.ascontiguousarray(hs.reshape(T, H)).astype(np.float32, copy=False)

    logits = x @ rw.astype(np.float32)  # [T, E]
    order = np.argsort(-logits, axis=1, kind="stable")[:, :2]
    l12 = np.take_along_axis(logits, order, axis=1).astype(np.float64)
    w1 = 1.0 / (1.0 + np.exp(l12[:, 1] - l12[:, 0]))  # renormalized top-2
    w2 = 1.0 - w1
    i1, i2 = order[:, 0], order[:, 1]

    idx, wts = [], []
    for e in range(E):
        m1 = i1 == e
        ide = np.nonzero(m1 | (i2 == e))[0]
        we = np.where(m1[ide], w1[ide], w2[ide]).astype(np.float32)
        idx.append(ide)
        wts.append(we)
    counts = np.array([len(v) for v in idx])

    # Split each expert's tokens into (n_slots/2) equal chunks -> 8*n_slots
    # chunks; assign the j-th largest octile of chunks to slot j of the 8
    # cores, so each slot's capacity hugs its octile's max and padded
    # compute is minimized.
    pieces = max(1, n_slots // 2)  # E=16 experts over 8 cores: n_slots/2 pieces each
    chunks = []  # (size, expert, start)
    for e in range(E):
        n = int(counts[e])
        st = 0
        for p in range(pieces):
            sz = (n - st + (pieces - p - 1)) // (pieces - p)  # even split
            chunks.append((sz, e, st))
            st += sz
        assert st == n
    chunks.sort(key=lambda t: (-t[0], t[1], t[2]))
    assert len(chunks) == N_CORES * n_slots

    caps = []
    for j in range(n_slots):
        mx = max(sz for sz, _, _ in chunks[N_CORES * j : N_CORES * (j + 1)])
        caps.append(max(64, _ceil_mult(mx, 4)))
    slots = [
        [chunks[N_CORES * j + c] for j in range(n_slots)] for c in range(N_CORES)
    ]

    mb = _OPT.get("mant_bits", 0)
    xT = _round_mant(np.ascontiguousarray(x.T).astype(NP_BF16), mb)  # [H, T]

    in_maps = []
    for c in range(N_CORES):
        m = {}
        for s, (sz, e, st) in enumerate(slots[c]):
            xe = np.zeros((H, caps[s]), NP_BF16)
            xe[:, :sz] = xT[:, idx[e][st : st + sz]]
            m[f"x{s}"] = xe
            m[f"g{s}"] = _round_mant(np.ascontiguousarray(gw[e]).astype(NP_BF16), mb)
            m[f"u{s}"] = _round_mant(np.ascontiguousarray(uw[e]).astype(NP_BF16), mb)
            m[f"d{s}"] = _round_mant(np.ascontiguousarray(dw[e]).astype(NP_BF16), mb)
        in_maps.append(m)

    meta = dict(
        B=B, S=S, T=T, idx=idx, wts=wts, counts=counts,
        slots=slots, caps=tuple(caps),
    )
    return in_maps, meta


def _combine(results, meta):
    """Host-side top-2 weighted combine (unshard)."""
    T = meta["T"]
    out = np.zeros((T, H), np.float32)
    for c in range(N_CORES):
        for s, (sz, e, st) in enumerate(meta["slots"][c]):
            if sz == 0:
                continue
            y = results[c][f"y{s}"][:, :sz].astype(np.float32)  # [H, sz]
            ide = meta["idx"][e][st : st + sz]
            out[ide] += meta["wts"][e][st : st + sz, None] * y.T
    return out.reshape(meta["B"], meta["S"], H)


def _run_spmd(nc, in_maps):
    try:
        return bass_utils.run_bass_kernel_spmd(
            nc, in_maps, core_ids=list(range(N_CORES))
        )
    except ModuleNotFoundError:
        # axon NTFF profiling hook unavailable in this container; retry
        # with tracing force-disabled.
        os.environ["BASS_NEVER_TRACE"] = "1"
        try:
            return bass_utils.run_bass_kernel_spmd(
                nc, in_maps, core_ids=list(range(N_CORES))
            )
        finally:
            os.environ.pop("BASS_NEVER_TRACE", None)


def kernel(hidden_states, router_w, gate_w, up_w, down_w):
    hs = np.asarray(hidden_states)
    rw = np.asarray(router_w)
    gw = np.asarray(gate_w)
    uw = np.asarray(up_w)
    dw = np.asarray(down_w)

    in_maps, meta = _route_and_shard(hs, rw, gw, uw, dw)
    nc = _get_nc(meta["caps"])
    res = _run_spmd(nc, in_maps)
    return _combine(res.results, meta)

